# revision 24
# baseline (speedup 1.0000x reference)
"""CenterLoss update kernel for 8 TRN2 NeuronCores (Bass, SPMD, collective-free).

Reference computation:
    embeded_labels = labels @ center          # one-hot gather   [N, D]
    diff           = embeded_labels - preds   #                  [N, D]
    grad           = labels.T @ diff          # scatter-add      [C, D]
    out            = center - 0.5 * grad

Algebraic rewrite (labels is one-hot per row, labels.T @ labels = diag(count)):
    grad[c] = count_c * center[c] - (labels.T @ preds)[c]
    out[c]  = (1 - 0.5*count_c) * center[c] + 0.5 * (labels.T @ preds)[c]

So the whole problem reduces to one matmul  S = labels.T @ [0.5*preds | 0.5]
([C, 257]; column 256 carries 0.5*count) plus a cheap per-row affine update.
No gather of center rows is needed at all.

Sharding: class-parallel. Core k owns classes [k*1250, (k+1)*1250) (padded to
1280): it reads its 1280-column shard of labels (the dominant tensor), all of
preds (replicated), and its 1280-row shard of center, and writes its shard of
the updated center. Zero device collectives; the host concatenates the 8
shard outputs.

Precision/layout choices:
  - fp32 matmuls on TRN2 decompose into LOW/HIGH passes (4 cyc/col measured),
    which made the fp32 version PE-bound at ~330 us. The matmul operands are
    therefore fed as bf16: one-hot labels are EXACTLY representable in bf16
    (zero information loss), and bf16 preds cost ~1.7e-3 relative error on
    the output, far under the 2e-2 gate. PSUM accumulation stays fp32, and
    the center/update path is pure fp32.
  - All device tensors are PRE-TILED on the host into [128, free] partition
    layout so every DMA is a fully contiguous burst per partition
    (~416 GB/s measured on the HWDGE path).
  - The batch is processed in groups of 128-row tiles; per group g and class
    tile ct, one matmul per batch tile accumulates
    labels[128b,128c].T @ preds_aug[128b,257] into a PSUM bank (4-bank
    rotation); VectorE folds banks into a per-class-tile fp32 SBUF
    accumulator and computes the final update, interleaved with the last
    group. The first two groups are half-sized so the TensorEngine starts
    as soon as ~1.3 MB of labels has landed.
"""

import os

import numpy as np

import concourse.bass as bass
import concourse.mybir as mybir
from concourse.bass_utils import run_bass_kernel_spmd

# Problem shape (hardcoded; kernel.py must be self-contained).
B = 8192          # batch
C = 10000         # num classes
D = 256           # num features
NCORES = 8
CPC = C // NCORES        # classes per core (1250)
CPAD = 1280              # padded classes per core (10 tiles of 128)
DA = D + 1               # preds augmented with the count column (257)
P = 128                  # partitions
CT = CPAD // P           # class tiles per core (10)
NPS = 4                  # PSUM banks rotated
NBUF = 3                 # label/preds SBUF buffer slots (sized for 8 tiles)
GMAX = 8                 # max batch tiles per group
# group sizes (batch tiles per group): small lead-in groups up front so the
# TensorEngine starts as soon as ~0.7 MB of labels has landed, then full
# groups. sum(GROUPS)*128 == B. All cumulative semaphore values stay <= 255.
GROUPS = [2, 2, 4] + [8] * 7
NG = len(GROUPS)


def build_nc() -> bass.Bass:
    nc = bass.Bass("TRN2")
    f32 = mybir.dt.float32
    bf16 = mybir.dt.bfloat16

    # Flat pre-tiled parameters (host lays out [128, free] per group).
    labels = nc.declare_dram_parameter("labels", [B * CPAD], bf16, isOutput=False)
    preds = nc.declare_dram_parameter("preds", [B * DA], bf16, isOutput=False)
    center = nc.declare_dram_parameter("center", [P, CT * D], f32, isOutput=False)
    out = nc.declare_dram_parameter("out", [P, CT * D], f32, isOutput=True)

    # per-group DRAM access patterns: [128, sz*width] contiguous per partition
    lab_aps, prd_aps = [], []
    lofs = pofs = 0
    for sz in GROUPS:
        lab_aps.append(
            labels[lofs : lofs + P * sz * CPAD].rearrange("(p x) -> p x", p=P)
        )
        prd_aps.append(
            preds[pofs : pofs + P * sz * DA].rearrange("(p x) -> p x", p=P)
        )
        lofs += P * sz * CPAD
        pofs += P * sz * DA

    with (
        nc.sbuf_tensor("lab", [P, NBUF, GMAX * CPAD], bf16) as lab,  # 60 KB/part
        nc.sbuf_tensor("prd", [P, NBUF, GMAX * DA], bf16) as prd,    # 12 KB/part
        nc.sbuf_tensor("acc", [P, CT, DA], f32) as acc,              # 10 KB/part
        nc.sbuf_tensor("cen", [P, CT, D], f32) as cen,               # 10 KB/part
        nc.sbuf_tensor("outb", [P, CT, D], f32) as outb,             # 10 KB/part
        nc.psum_tensor("ps", [P, NPS, 512], f32) as ps,
        nc.semaphore("lab_sem") as lab_sem,
        nc.semaphore("prd_sem") as prd_sem,
        nc.semaphore("cen_sem") as cen_sem,
        nc.semaphore("mm_sem") as mm_sem,
        nc.semaphore("ev_sem") as ev_sem,
        nc.semaphore("upd_sem") as upd_sem,
        nc.semaphore("out_sem") as out_sem,
        nc.Block() as block,
    ):
        @block.gpsimd
        def _(gpsimd):
            # preds travel on the SWDGE queue, in parallel with labels on
            # the sync/HWDGE queue (plain bf16 copy, no cast).
            for g in range(NG):
                if g >= NBUF:
                    gpsimd.wait_ge(mm_sem, (g - NBUF + 1) * CT)
                s = g % NBUF
                sz = GROUPS[g]
                gpsimd.dma_start(
                    out=prd[:, s, 0 : sz * DA], in_=prd_aps[g]
                ).then_inc(prd_sem, 16)

        @block.sync
        def _(sync):
            for g in range(NG):
                if g >= NBUF:
                    # slot g%NBUF is free once group g-NBUF's matmuls are done
                    sync.wait_ge(mm_sem, (g - NBUF + 1) * CT)
                s = g % NBUF
                sz = GROUPS[g]
                sync.dma_start(
                    out=lab[:, s, 0 : sz * CPAD], in_=lab_aps[g]
                ).then_inc(lab_sem, 16)
                if g == 2:
                    sync.dma_start(out=cen[:].rearrange("p t d -> p (t d)"),
                                   in_=center[:]).then_inc(cen_sem, 16)
            # output in 2-class-tile chunks overlapping the tail updates
            for j in range(CT // 2):
                sync.wait_ge(upd_sem, 2 * (j + 1))
                sync.dma_start(
                    out=out[:, 2 * j * D : 2 * (j + 1) * D],
                    in_=outb[:, 2 * j : 2 * (j + 1)].rearrange("p t d -> p (t d)"),
                ).then_inc(out_sem, 16)
            sync.wait_ge(out_sem, 16 * (CT // 2))

        @block.tensor
        def _(tensor):
            for g in range(NG):
                tensor.wait_ge(lab_sem, 16 * (g + 1))
                tensor.wait_ge(prd_sem, 16 * (g + 1))
                s = g % NBUF
                sz = GROUPS[g]
                for ct in range(CT):
                    i = g * CT + ct
                    if i >= NPS:
                        tensor.wait_ge(ev_sem, i - NPS + 1)
                    pb = ps[:, i % NPS, 0:DA]
                    mm = None
                    for bt in range(sz):
                        mm = tensor.matmul(
                            pb,
                            lab[:, s, bt * CPAD + ct * P : bt * CPAD + (ct + 1) * P],
                            prd[:, s, bt * DA : (bt + 1) * DA],
                            start=(bt == 0),
                            stop=(bt == sz - 1),
                        )
                    mm.then_inc(mm_sem, 1)

        @block.vector
        def _(vector):
            # out = center - center*(0.5*count) + 0.5*scatter, computed as
            # three elementwise ops per tile. The broadcast (free-step-0)
            # operand acc[:, ct, 256] must be read only well after it was
            # written: DVE broadcast/scalar reads fetch early relative to the
            # previous op's writeback, so a distance-1 same-engine RAW on a
            # broadcast source returns stale data. Updates are therefore
            # interleaved two tiles behind the final group's evictions
            # (>= 2 ops / ~1 us of separation). The distance-1 RAW on outb
            # is elementwise in matching stream order, which is safe.
            def update(ct):
                vector.tensor_tensor(
                    out=outb[:, ct, :],
                    in0=cen[:, ct, :],
                    in1=acc[:, ct, D : D + 1].to_broadcast([P, D]),
                    op=mybir.AluOpType.mult,
                )
                vector.tensor_tensor(
                    out=outb[:, ct, :],
                    in0=cen[:, ct, :],
                    in1=outb[:, ct, :],
                    op=mybir.AluOpType.subtract,
                )
                vector.tensor_tensor(
                    out=outb[:, ct, :],
                    in0=outb[:, ct, :],
                    in1=acc[:, ct, 0:D],
                    op=mybir.AluOpType.add,
                ).then_inc(upd_sem, 1)

            for g in range(NG):
                last = g == NG - 1
                for ct in range(CT):
                    i = g * CT + ct
                    vector.wait_ge(mm_sem, i + 1)
                    pb = ps[:, i % NPS, 0:DA]
                    if g == 0:
                        vector.tensor_copy(acc[:, ct, :], pb).then_inc(ev_sem, 1)
                    else:
                        vector.tensor_tensor(
                            out=acc[:, ct, :],
                            in0=acc[:, ct, :],
                            in1=pb,
                            op=mybir.AluOpType.add,
                        ).then_inc(ev_sem, 1)
                    if last:
                        if ct == 1:
                            vector.wait_ge(cen_sem, 16)
                        if ct >= 2:
                            update(ct - 2)
            update(CT - 2)
            update(CT - 1)

    return nc


def _tile_rows(a, sizes, width):
    """Rows [N, width] -> flat pre-tiled [(group, partition, tile, col)]."""
    blocks = []
    base = 0
    for sz in sizes:
        blk = a[base : base + sz * P]
        # [sz*128, width] -> [128, sz*width] with row t*128+p on partition p
        blocks.append(
            blk.reshape(sz, P, width).transpose(1, 0, 2).reshape(P, sz * width)
        )
        base += sz * P
    return np.concatenate([b.reshape(-1) for b in blocks])


def _shard_inputs(embeded_preds, labels, center):
    import ml_dtypes

    bf16 = ml_dtypes.bfloat16
    embeded_preds = np.ascontiguousarray(embeded_preds, dtype=np.float32)
    labels = np.ascontiguousarray(labels, dtype=np.float32)
    center = np.ascontiguousarray(center, dtype=np.float32)

    # preds_aug pre-scaled by the 0.5 learning rate, with a 0.5 count column
    pa = np.empty((B, DA), dtype=np.float32)
    pa[:, :D] = embeded_preds
    pa[:, :D] *= 0.5
    pa[:, D] = 0.5
    pa_tiled = _tile_rows(pa.astype(bf16), GROUPS, DA)

    in_maps = []
    for k in range(NCORES):
        lab = np.zeros((B, CPAD), dtype=bf16)
        lab[:, :CPC] = labels[:, k * CPC : (k + 1) * CPC].astype(bf16)
        lab_tiled = _tile_rows(lab, GROUPS, CPAD)
        cenk = np.zeros((CPAD, D), dtype=np.float32)
        cenk[:CPC] = center[k * CPC : (k + 1) * CPC]
        cen_tiled = cenk.reshape(CT, P, D).transpose(1, 0, 2).reshape(P, CT * D)
        in_maps.append(
            {"labels": lab_tiled, "preds": pa_tiled, "center": cen_tiled}
        )
    return in_maps


def kernel(embeded_preds, labels, center):
    in_maps = _shard_inputs(embeded_preds, labels, center)
    nc = build_nc()

    trace = os.environ.get("KERNEL_TRACE") == "1"
    kwargs = {}
    if trace:
        try:
            import ntff_shim

            ntff_shim.install()
        except Exception as e:  # profiling is best-effort; results still valid
            print(f"ntff shim unavailable: {e}")
        tdir = os.environ.get("KERNEL_TRACE_DIR")
        if tdir:
            kwargs["tmpdir"] = tdir

    res = run_bass_kernel_spmd(
        nc, in_maps, core_ids=list(range(NCORES)), trace=trace, **kwargs
    )
    if trace:
        print(f"HW exec time: {res.exec_time_ns} ns")

    # un-tile each core's [128, CT*D] output back to [CPAD, D] rows
    shards = []
    for k in range(NCORES):
        o = res.results[k]["out"]
        shards.append(
            o.reshape(P, CT, D).transpose(1, 0, 2).reshape(CPAD, D)[:CPC]
        )
    return np.ascontiguousarray(np.concatenate(shards, axis=0), dtype=np.float32)
